# revision 12
# baseline (speedup 1.0000x reference)
"""Trainium2 Bass kernel for nn_Attention (batch=8, seq=1024, dim=512, 8 heads x 64).

Strategy: pure data parallelism — one batch element per NeuronCore (8 cores).
No collectives. Per core, everything is computed from a pre-transposed
x^T [512, 1024] so all matmul contractions sit on the partition axis:

  Q^T = wq @ x^T, K^T = wk @ x^T          (d-major, per-head rows)
  S^T[nk, nq] = (K^T)_h-slices.T @ (Q^T)_h  (K=64 contraction; the two heads
                                             of a pair run on disjoint PE row
                                             halves -> concurrent matmuls)
  E^T = clip(exp(S/8), e^1e-6, e^1)         (exp on ACT evicts PSUM->SBUF,
                                             then DVE/GpSimd clip)
  O^T_aug = [V | ones64].T @ E^T            (64 ones-columns per head yield the
                                             softmax rowsum PRE-BROADCAST on
                                             PSUM partitions 64..127)
  O^T = O^T_aug[0:64] * recip(O^T_aug[64:128])   (DVE reciprocal_approx_fast +
                                             tensor_mul; no ACT, no DMA bounce)
  y^T = woT-slices.T @ O^T-slices + bo      (final proj computed transposed so
                                             the bias is per-partition; ACT
                                             Copy-activation adds it on evict;
                                             host transposes the output back)

Weights/x are cast to bf16 and transposed on the host; accumulation is f32.
"""

import numpy as np
import concourse.bass as bass
import concourse.tile as tile
from concourse import mybir
from concourse.bass_utils import run_bass_kernel_spmd

F32 = mybir.dt.float32
BF16 = mybir.dt.bfloat16

DIM = 512
HEADS = 8
DH = 64
N = 1024
NCORES = 8
SCALE = DH**-0.5
E_LO = float(np.exp(1e-6))
E_HI = float(np.exp(1.0))
EXP = mybir.ActivationFunctionType.Exp
LN = mybir.ActivationFunctionType.Ln
IDENT = mybir.ActivationFunctionType.Identity
MIN = mybir.AluOpType.min
MAX = mybir.AluOpType.max


def split_multiwait(nc, max_waits=1):
    """Walrus in this env rejects instructions carrying more than one sync
    wait ("Too many sync wait commands" in setupSyncWait). Tile's tail drain
    legitimately accumulates several; split the excess into single-wait NOPs
    inserted just before the offending instruction."""
    nsplit = 0
    for fn in nc.m.functions:
        for bb in fn.blocks:
            insts = list(bb.instructions)
            if not any(
                i.sync_info is not None and len(i.sync_info.on_wait) > max_waits
                for i in insts
            ):
                continue
            new = []
            for i in insts:
                si = i.sync_info
                if si is not None and len(si.on_wait) > max_waits:
                    waits = list(si.on_wait)
                    splittable = [w for w in waits if w.wait_reg is None]
                    keep = [w for w in waits if w.wait_reg is not None]
                    nkeep = max_waits - len(keep)
                    assert nkeep >= 0, "too many register waits to split"
                    tail = splittable[-nkeep:] if nkeep > 0 else []
                    head = splittable[: len(splittable) - len(tail)]
                    for k, w in enumerate(head):
                        nop = mybir.InstNoOp(name=f"{i.name}-sw{k}")
                        nop.engine = i.engine
                        nop.sync_info = mybir.SyncInfo(on_wait=[w], on_update=[])
                        new.append(nop)
                        nsplit += 1
                    i.sync_info = mybir.SyncInfo(
                        on_wait=keep + tail, on_update=list(si.on_update)
                    )
                new.append(i)
            bb.instructions.clear()
            for i in new:
                bb.add_instruction(i)
    return nsplit


def build_nc(et_bufs=48, n_warmup=18):
    nc = bass.Bass("TRN2")
    xT = nc.dram_tensor("xT", [DIM, N], BF16, kind="ExternalInput")
    wqT = nc.dram_tensor("wqT", [DIM, DIM], BF16, kind="ExternalInput")
    wkT = nc.dram_tensor("wkT", [DIM, DIM], BF16, kind="ExternalInput")
    wvT = nc.dram_tensor("wvT", [DIM, DIM], BF16, kind="ExternalInput")
    woT = nc.dram_tensor("woT", [DIM, DIM], BF16, kind="ExternalInput")
    bopm = nc.dram_tensor("bopm", [128, 4], F32, kind="ExternalInput")
    outT = nc.dram_tensor("outT", [DIM, N], F32, kind="ExternalOutput")

    with tile.TileContext(nc) as tc:
        with (
            tc.tile_pool(name="consts", bufs=1) as consts,
            tc.tile_pool(name="etp", bufs=et_bufs) as etp,
            tc.tile_pool(name="rp", bufs=4) as rp,
            tc.tile_pool(name="yp", bufs=4) as yp,
            tc.tile_pool(name="pp_st", bufs=2, space="PSUM") as pp_st,
            tc.tile_pool(name="pp_oa", bufs=2, space="PSUM") as pp_oa,
        ):
            # ---- PE warm-up + ACT table preload ---------------------------
            # A junk-matmul stream keeps the PE HAM activity window busy
            # through the input-DMA phase so real matmuls run at 2.4 GHz;
            # a tiny exp triggers the one-time ACT_TABLE_LOAD up front.
            wu = consts.tile([128, 512], BF16, name="wu", tag="wu")
            nc.gpsimd.memset(wu, 0.0)
            wu_ps = pp_st.tile([128, 512], F32, name="wu_ps", tag="st")
            for _ in range(n_warmup):
                nc.tensor.matmul(wu_ps, lhsT=wu[:, 0:128], rhs=wu, start=True, stop=True)

            # ---- constant loads -------------------------------------------
            # xt gates everything -> dedicated sync ring; wq/wk split across
            # the scalar/gpsimd rings ahead of the late-needed wv/wo.
            def load4(src, cols, nm, eng):
                tiles = []
                for k in range(4):
                    t = consts.tile([128, cols], BF16, name=f"{nm}{k}", tag=f"{nm}{k}")
                    eng.dma_start(t, src[k * 128 : (k + 1) * 128, :])
                    tiles.append(t)
                return tiles

            xt = load4(xT, N, "xt", nc.sync)
            wq = load4(wqT, DIM, "wq", nc.scalar)
            wk = load4(wkT, DIM, "wk", nc.gpsimd)
            wv = load4(wvT, DIM, "wv", nc.gpsimd)
            wo = load4(woT, DIM, "wo", nc.gpsimd)
            bo_sb = consts.tile([128, 4], F32, name="bo_sb", tag="bo_sb")
            nc.sync.dma_start(bo_sb, bopm[:, :])
            wu_e = consts.tile([128, 1], BF16, name="wu_e", tag="wu_e")
            nc.scalar.activation(wu_e, wu[:, 0:1], EXP, scale=SCALE)

            # V-augmented tiles: per j-tile of keys, per head h the 128-col
            # block [V_h | ones64]; the ones never change, set them now
            # (during the DMA phase) and only write the V half later.
            vaug = []
            for j in range(8):
                va = consts.tile([128, HEADS * 128], BF16, name=f"va{j}", tag=f"va{j}")
                va3 = va.rearrange("p (h c) -> p h c", c=128)
                nc.gpsimd.memset(va3[:, :, DH : 2 * DH], 1.0)
                vaug.append(va)

            qT, kT = [None] * 4, [None] * 4
            et = {}
            oT = []
            for p in range(4):
                o = consts.tile([128, N], BF16, name=f"oT{p}", tag=f"oT{p}")
                oT.append(o)

            def proj_qk(p):
                # k-outer / c-inner so each LDWEIGHTS covers two matmuls; the
                # two c-halves share one [128,1024] PSUM tile -> one eviction.
                q = consts.tile([128, N], BF16, name=f"qT{p}", tag=f"qT{p}")
                k_ = consts.tile([128, N], BF16, name=f"kT{p}", tag=f"kT{p}")
                qT[p], kT[p] = q, k_
                for wsb, dst in ((wq, q), (wk, k_)):
                    ps = pp_st.tile([128, N], F32, name="ps_proj", tag="st")
                    for k in range(4):
                        lhsT = wsb[k][:, p * 128 : (p + 1) * 128]
                        nc.tensor.matmul(
                            ps[:, 0:512], lhsT=lhsT, rhs=xt[k][:, 0:512],
                            start=(k == 0), stop=(k == 3),
                        )
                        nc.tensor.matmul(
                            ps[:, 512:1024], lhsT=lhsT, rhs=xt[k][:, 512:1024],
                            start=(k == 0), stop=(k == 3),
                        )
                    nc.vector.tensor_copy(dst, ps)

            def proj_v(j):
                ps = pp_st.tile([128, 512], F32, name="ps_vproj", tag="st")
                for k in range(4):
                    nc.tensor.matmul(
                        ps,
                        lhsT=xt[k][:, j * 128 : (j + 1) * 128],
                        rhs=wv[k],
                        start=(k == 0),
                        stop=(k == 3),
                    )
                va3 = vaug[j].rearrange("p (h c) -> p h c", c=128)
                nc.vector.tensor_copy(
                    va3[:, :, 0:DH], ps.rearrange("p (h c) -> p h c", c=DH)
                )

            def st_pair(p, hook=None):
                # S^T for heads (2p, 2p+1): head hh=0 streams from SBUF/PE
                # partitions 0-63, hh=1 from 64-127 -> disjoint PE row halves
                # run concurrently. exp (ACT) evicts PSUM->SBUF, then clip.
                # hook(j) interleaves other work into the exp-paced stream.
                for j in range(8):
                    pss = []
                    for hh in range(2):
                        ps = pp_st.tile([128, N], F32, name="ps_st", tag="st")
                        lhsT = kT[p][hh * 64 : (hh + 1) * 64, j * 128 : (j + 1) * 128]
                        for c in range(2):
                            nc.tensor.matmul(
                                ps[:, c * 512 : (c + 1) * 512],
                                lhsT=lhsT,
                                rhs=qT[p][hh * 64 : (hh + 1) * 64, c * 512 : (c + 1) * 512],
                                start=True,
                                stop=True,
                            )
                        pss.append(ps)
                    for hh in range(2):
                        h = 2 * p + hh
                        e = etp.tile([128, N], BF16, name="et", tag="et")
                        nc.scalar.activation(e, pss[hh], EXP, scale=SCALE)
                        ceng = nc.gpsimd if hh else nc.vector
                        ceng.tensor_scalar(e, e, E_HI, E_LO, MIN, MAX)
                        et[(h, j)] = e
                    if hook is not None:
                        hook(j)

            # oa tiles per in-flight pair: oah[hh] = [128,1024] (c0|c1 halves),
            # rows 0..63 = unnormalized O^T, rows 64..127 = rowsum broadcast.
            oah = {}

            def attn_j(p, hh, j):
                # One j-step of the O^T_aug accumulation for head 2p+hh.
                h = 2 * p + hh
                if j == 0:
                    oah[hh] = pp_oa.tile([128, N], F32, name="ps_oa", tag="oa")
                lhsT = vaug[j][:, h * 128 : (h + 1) * 128]
                for c in range(2):
                    nc.tensor.matmul(
                        oah[hh][:, c * 512 : (c + 1) * 512],
                        lhsT=lhsT,
                        rhs=et[(h, j)][:, c * 512 : (c + 1) * 512],
                        start=(j == 0),
                        stop=(j == 7),
                    )

            def attn_mms(p, hh):
                for j in range(8):
                    attn_j(p, hh, j)

            def norm(p):
                # reciprocal = exp(-ln r) on ACT (ln and exp share one table
                # set -> no table reload), then one DVE tensor_mul per head.
                lr = rp.tile([128, N], F32, name="lr", tag="lr")
                nc.scalar.activation(lr[0:64, :], oah[0][64:128, :], LN)
                nc.scalar.activation(lr[64:128, :], oah[1][64:128, :], LN)
                rinv = rp.tile([128, N], F32, name="rinv", tag="rinv")
                nc.scalar.activation(rinv, lr, EXP, scale=-1.0)
                nc.vector.tensor_mul(oT[p][0:64, :], oah[0][0:64, :], rinv[0:64, :])
                nc.vector.tensor_mul(
                    oT[p][64:128, :], oah[1][0:64, :], rinv[64:128, :]
                )

            def final(c):
                # y^T[dt-rows, c-half] = sum_k woT_k-slices.T @ oT[k], bias
                # added per-partition by the ACT Identity eviction.
                for dt in range(4):
                    pool = pp_st if dt % 2 == 0 else pp_oa
                    tag = "st" if dt % 2 == 0 else "oa"
                    ps = pool.tile([128, 512], F32, name="ps_fin", tag=tag)
                    for k in range(4):
                        nc.tensor.matmul(
                            ps,
                            lhsT=wo[k][:, dt * 128 : (dt + 1) * 128],
                            rhs=oT[k][:, c * 512 : (c + 1) * 512],
                            start=(k == 0),
                            stop=(k == 3),
                        )
                    y = yp.tile([128, 512], F32, name="y", tag="y")
                    nc.scalar.activation(y, ps, IDENT, bias=bo_sb[:, dt : dt + 1])
                    nc.sync.dma_start(
                        outT[dt * 128 : (dt + 1) * 128, c * 512 : (c + 1) * 512], y
                    )

            # ---- interleaved emission --------------------------------------
            # S^T(0) right after its own Q/K projection so the ACT exp cadence
            # (the critical resource) starts early. attn matmuls for pair p-1
            # are hooked at the FRONT of pair p's S/exp stream (clips are all
            # done, PE has slack there) and the ln/exp reciprocal mid-stream;
            # pair 3's O accumulation runs per-j in its own stream so only a
            # small remainder trails the last exp.
            def hook0(j):
                if j == 2:
                    proj_qk(1)
                elif j >= 4:
                    proj_v(2 * (j - 4))
                    proj_v(2 * (j - 4) + 1)

            def mk_hook(p):
                def hook(j):
                    if j == 0:
                        attn_mms(p - 1, 0)
                    elif j == 1:
                        attn_mms(p - 1, 1)
                    elif j == 3:
                        norm(p - 1)
                    elif j == 5 and p + 1 <= 3:
                        proj_qk(p + 1)
                return hook

            def hook3(j):
                if j == 0:
                    attn_mms(2, 0)
                elif j == 1:
                    attn_mms(2, 1)
                elif j == 2:
                    norm(2)
                elif j >= 4:
                    # per-j partial O for pair 3, two et-steps per hook
                    for jj in (2 * (j - 4), 2 * (j - 4) + 1):
                        if jj <= 6:
                            attn_j(3, 0, jj)
                            attn_j(3, 1, jj)

            proj_qk(0)
            st_pair(0, hook0)
            st_pair(1, mk_hook(1))
            st_pair(2, mk_hook(2))
            st_pair(3, hook3)
            attn_j(3, 0, 7)
            attn_j(3, 1, 7)
            norm(3)
            final(0)
            final(1)

    split_multiwait(nc)
    return nc


_NC = None


def _get_nc():
    global _NC
    if _NC is None:
        _NC = build_nc()
    return _NC


def make_in_maps(x, wq, wk, wv, wo, bo):
    bf = mybir.dt.np(BF16)
    shared = {
        "wqT": np.ascontiguousarray(wq.T).astype(bf),
        "wkT": np.ascontiguousarray(wk.T).astype(bf),
        "wvT": np.ascontiguousarray(wv.T).astype(bf),
        "woT": np.ascontiguousarray(wo.T).astype(bf),
        "bopm": np.ascontiguousarray(
            np.asarray(bo, dtype=np.float32).reshape(4, 128).T
        ),
    }
    xT_all = np.ascontiguousarray(x.transpose(0, 2, 1)).astype(bf)
    return [{"xT": xT_all[b], **shared} for b in range(NCORES)]


def run(x, wq, wk, wv, wo, bo, **spmd_kwargs):
    nc = _get_nc()
    in_maps = make_in_maps(
        np.asarray(x), np.asarray(wq), np.asarray(wk),
        np.asarray(wv), np.asarray(wo), np.asarray(bo),
    )
    res = run_bass_kernel_spmd(nc, in_maps, core_ids=list(range(NCORES)), **spmd_kwargs)
    out = np.stack(
        [np.ascontiguousarray(res.results[b]["outT"].T) for b in range(NCORES)], axis=0
    )
    return out.astype(np.float32), res


def kernel(x, wq, wk, wv, wo, bo):
    out, _ = run(x, wq, wk, wv, wo, bo)
    return out
